# revision 21
# baseline (speedup 1.0000x reference)
"""MoE block (B=4, T=1024, D=1024, H=4096, E=8, top-2) on 8 Trainium2 cores.

Strategy: expert-parallel. The reference computes every expert for every
token and then keeps only the top-2; the output only depends on the top-2
selections, so we route: host computes the (tiny) gating in float64, core e
runs the dense FFN for expert e over just the tokens routed to it
(transposed layout, zero on-device transposes), and the host combines the
two selected expert outputs per token with the gate weights.

Device kernel per core (C = max tokens routed to any expert, padded):
    h1T[H, C]  = relu(w1[e].T @ xT + b1)   (bf16 in, fp32 accum)
    yT [D, C]  = w2[e].T @ h1T + b2
All matmuls run in bf16 at the full PE rate; PE-roofline is ~C*12288 cycles.
"""

import numpy as np
import ml_dtypes

B, T, D, H, E = 4, 1024, 1024, 4096, 8
TOP_K = 2
N_CORES = 8
KD = D // 128   # 8  K-tiles over D
KH = H // 128   # 32 K-tiles over H

_PROGRAM_CACHE: dict = {}


def _build_program(C: int):
    """Bass/Tile program: dense FFN for one expert over C token columns."""
    if C in _PROGRAM_CACHE:
        return _PROGRAM_CACHE[C]

    from contextlib import ExitStack
    import concourse.bacc as bacc
    import concourse.mybir as mybir
    import concourse.tile as tile

    bf = mybir.dt.bfloat16
    f32 = mybir.dt.float32
    AFT = mybir.ActivationFunctionType

    nc = bacc.Bacc("TRN2", target_bir_lowering=False, debug=False,
                   num_devices=N_CORES)

    A = min(512, C)          # first column block, DMA'd first
    GB = C - A               # remaining columns
    G = 2                    # w1 m-tiles per DMA group
    xa_d = nc.dram_tensor("xa", [128, KD * A], bf, kind="ExternalInput")
    if GB:
        xb_d = nc.dram_tensor("xb", [128, KD * GB], bf, kind="ExternalInput")
    w1_d = nc.dram_tensor("w1g", [KH // G, 128, G * D], bf,
                          kind="ExternalInput")
    w2_d = nc.dram_tensor("w2q", [KD, 128, H], bf, kind="ExternalInput")
    b1_d = nc.dram_tensor("b1t", [128, KH], f32, kind="ExternalInput")
    b2_d = nc.dram_tensor("b2t", [128, KD], f32, kind="ExternalInput")
    y_d = nc.dram_tensor("y", [D, C], f32, kind="ExternalOutput")
    dump_d = nc.dram_tensor("dump", [1, 4], f32, kind="ExternalOutput")

    chunks = [(s, min(512, C - s)) for s in range(0, C, 512)]
    # phase A column passes: pass 0 covers [0,A) from xa, pass 1 covers
    # [A,C) from xb (sub-chunked to <=512-wide matmuls)
    a_passes = [(0, 0, A, [(0, A)] if A else [])]
    if GB:
        a_passes.append((A, 1, GB,
                         [(s, min(512, GB - s)) for s in range(0, GB, 512)]))

    with tile.TileContext(nc) as tc, ExitStack() as ctx:
        pers = ctx.enter_context(tc.tile_pool(name="pers", bufs=1))
        wpool = ctx.enter_context(tc.tile_pool(name="w", bufs=2))
        opool = ctx.enter_context(tc.tile_pool(name="o", bufs=2))
        psum = ctx.enter_context(tc.tile_pool(name="ps", bufs=2, space="PSUM"))
        psum_w = ctx.enter_context(tc.tile_pool(name="psw", bufs=1,
                                                space="PSUM"))

        # Startup-critical DMAs are issued from otherwise-idle engines
        # BEFORE anything else enters those engines' in-order streams:
        # every dma_start costs ~0.6us of serial descriptor processing on
        # its issuing engine, and SyncE additionally has a ~6.5us prologue.
        # The first w1 group goes first so the first matmul can issue as
        # soon as xa lands; xb (the remaining token columns) streams in
        # behind while the first column pass computes.
        w1gs = [wpool.tile([128, G * D], bf, tag="w1g", name=f"w1g{g}",
                           bufs=3) for g in range(3)]
        nc.scalar.dma_start(w1gs[0][:], w1_d[0])
        xa = pers.tile([128, KD * A], bf)
        nc.scalar.dma_start(xa[:], xa_d[:])
        # same (FIFO) scalar ring: these trail xa instead of competing
        nc.scalar.dma_start(w1gs[1][:], w1_d[1])
        nc.scalar.dma_start(w1gs[2][:], w1_d[2])
        b1s = pers.tile([128, KH], f32)
        nc.gpsimd.dma_start(b1s[:], b1_d[:])
        b2s = pers.tile([128, KD], f32)
        nc.gpsimd.dma_start(b2s[:], b2_d[:])
        xb_dma = None
        if GB:
            xb = pers.tile([128, KD * GB], bf)
            xb_dma = nc.gpsimd.dma_start(xb[:], xb_d[:])

        # PE warmup: dummy matmuls during the initial DMA window keep the
        # PE HAM activity monitor busy so the clock gate opens (1.2 ->
        # 2.4 GHz) before the real matmuls start. The result is copied on
        # the (otherwise idle) vector engine and dumped to a throwaway
        # output so nothing optimizes it away; neither op blocks the DMA
        # descriptor streams above.
        wu = pers.tile([128, 128], bf)
        nc.vector.memset(wu[:], 0.0)
        wacc = psum_w.tile([128, 128], f32, tag="warm")
        for _ in range(36):
            nc.tensor.matmul(wacc[:], wu[:], wu[:], start=True, stop=True)
        wout = pers.tile([1, 4], f32)
        nc.vector.tensor_copy(wout[:], wacc[0:1, 0:4])
        nc.gpsimd.dma_start(dump_d[:], wout[:])

        h1s = [pers.tile([128, C], bf, tag=f"h1_{m}", name=f"h1_{m}")
               for m in range(KH)]

        # Phase A: h1T[m][cols] = relu(w1.T @ xT + b1), column-pass-outer
        # so pass 0 only needs xa. Weights stream in G-m-tile groups and
        # are re-fetched per pass (DMA has bandwidth to spare).
        relu_insts = []
        for (dst_s, src, srcw, subchunks) in a_passes:
            xsrc = xa if src == 0 else xb
            tot = sum(w for _, w in subchunks)
            for g in range(KH // G):
                if dst_s == 0 and g < 3:
                    w1g = w1gs[g]
                else:
                    w1g = wpool.tile([128, G * D], bf, tag="w1g", bufs=3)
                    nc.sync.dma_start(w1g[:], w1_d[g])
                for j in range(G):
                    m = g * G + j
                    acc = psum.tile([128, C], f32, tag="acc")
                    for k in range(KD):
                        for (so, w) in subchunks:
                            nc.tensor.matmul(
                                acc[:, so:so + w],
                                w1g[:, j * D + k * 128:j * D + (k + 1) * 128],
                                xsrc[:, k * srcw + so:k * srcw + so + w],
                                start=(k == 0), stop=(k == KD - 1))
                    relu_insts.append(
                        nc.scalar.activation(h1s[m][:, dst_s:dst_s + tot],
                                             acc[:, 0:tot], AFT.Relu,
                                             bias=b1s[:, m:m + 1]))

        # xb is not needed until the second column pass (~60us in); keep
        # its 1.2MB out of the startup rush.
        if xb_dma is not None:
            tile.add_dep_helper(relu_insts[1].ins, xb_dma.ins,
                                sync=True, reason="delay xb load")

        # Phase B: yT[d] = w2.T @ h1T + b2, d over D tiles
        for d in range(KD):
            w2s = wpool.tile([128, H], bf, tag="w2s", bufs=3)
            w2dma = nc.sync.dma_start(w2s[:], w2_d[d])
            if d < 3:
                # Keep the 1MB w2 prefetches out of the startup DMA rush:
                # their packets would otherwise delay xa/w1 and stall the
                # first matmuls.
                tile.add_dep_helper(relu_insts[2 + 2 * d].ins, w2dma.ins,
                                    sync=True, reason="delay w2 prefetch")
            acc = psum.tile([128, C], f32, tag="acc")
            last = (d == KD - 1)
            if not last:
                for k in range(KH):
                    for (s, w) in chunks:
                        nc.tensor.matmul(acc[:, s:s + w],
                                         w2s[:, k * 128:(k + 1) * 128],
                                         h1s[k][:, s:s + w],
                                         start=(k == 0), stop=(k == KH - 1))
                ost = opool.tile([128, C], f32, tag="ost")
                nc.scalar.activation(ost[:], acc[:], AFT.Identity,
                                     bias=b2s[:, d:d + 1])
                nc.sync.dma_start(y_d[d * 128:(d + 1) * 128, :], ost[:])
            else:
                # Last tile: finish chunk-by-chunk so the tail ACT+DMA
                # only covers the final chunk instead of the full row.
                for (s, w) in chunks:
                    for k in range(KH):
                        nc.tensor.matmul(acc[:, s:s + w],
                                         w2s[:, k * 128:(k + 1) * 128],
                                         h1s[k][:, s:s + w],
                                         start=(k == 0), stop=(k == KH - 1))
                    ost = opool.tile([128, C], f32, tag="ost")
                    nc.scalar.activation(ost[:, s:s + w], acc[:, s:s + w],
                                         AFT.Identity, bias=b2s[:, d:d + 1])
                    nc.sync.dma_start(y_d[d * 128:(d + 1) * 128, s:s + w],
                                      ost[:, s:s + w])

    nc.compile()
    _PROGRAM_CACHE[C] = nc
    return nc


def _route(x2, gate_w, gate_b):
    """Float64 gating: returns (top_idx [N,2], top_gate [N,2])."""
    logits = x2.astype(np.float64) @ gate_w.astype(np.float64) \
        + gate_b.astype(np.float64)
    z = np.exp(logits - logits.max(axis=1, keepdims=True))
    probs = z / z.sum(axis=1, keepdims=True)
    top = np.argsort(-logits, axis=1, kind="stable")[:, :TOP_K]
    gv = np.take_along_axis(probs, top, axis=1)
    return top, gv


def _pack_weights(w1e, w2e, b1e, b2e, G=2):
    bfl = ml_dtypes.bfloat16
    # w1g[g, p, (j, k, c)] = w1[e][k*128+p, (g*G+j)*128+c]
    w1q = w1e.reshape(KD, 128, KH, 128).transpose(2, 1, 0, 3)  # [m, p, k, c]
    w1g = np.ascontiguousarray(
        w1q.reshape(KH // G, G, 128, D).transpose(0, 2, 1, 3)
        .reshape(KH // G, 128, G * D).astype(bfl))
    w2q = np.ascontiguousarray(
        w2e.reshape(KH, 128, KD, 128).transpose(2, 1, 0, 3)
        .reshape(KD, 128, H).astype(bfl))
    b1t = np.ascontiguousarray(b1e.reshape(KH, 128).T.astype(np.float32))
    b2t = np.ascontiguousarray(b2e.reshape(KD, 128).T.astype(np.float32))
    return w1g, w2q, b1t, b2t


def kernel(x, gate_w, gate_b, w1, b1, w2, b2, _bench_hook=None):
    from concourse.bass_utils import run_bass_kernel_spmd

    bfl = ml_dtypes.bfloat16
    x = np.asarray(x, np.float32)
    x2 = x.reshape(-1, D)                       # [N, D], N = B*T
    N = x2.shape[0]

    top, gv = _route(x2, np.asarray(gate_w), np.asarray(gate_b))

    tok_lists = [np.where((top == e).any(axis=1))[0] for e in range(E)]
    maxload = max(1, max(len(t) for t in tok_lists))
    C = (maxload + 3) // 4 * 4

    nc = _build_program(C)

    x2b = x2.astype(bfl)
    A = min(512, C)
    GB = C - A
    in_maps = []
    for e in range(E):
        toks = tok_lists[e]
        xt = np.zeros((D, C), bfl)
        if len(toks):
            xt[:, :len(toks)] = x2b[toks].T
        xq = xt.reshape(KD, 128, C).transpose(1, 0, 2)   # [p, k, c]
        m = {"xa": np.ascontiguousarray(xq[:, :, :A]).reshape(128, KD * A)}
        if GB:
            m["xb"] = np.ascontiguousarray(xq[:, :, A:]).reshape(128, KD * GB)
        w1g, w2q, b1t, b2t = _pack_weights(
            np.asarray(w1[e], np.float32), np.asarray(w2[e], np.float32),
            np.asarray(b1[e], np.float32), np.asarray(b2[e], np.float32))
        m.update({"w1g": w1g, "w2q": w2q, "b1t": b1t, "b2t": b2t})
        in_maps.append(m)

    res = run_bass_kernel_spmd(nc, in_maps, core_ids=list(range(N_CORES)))
    if _bench_hook is not None:
        _bench_hook(nc, in_maps)

    out = np.zeros((N, D), np.float64)
    for e in range(E):
        toks = tok_lists[e]
        if not len(toks):
            continue
        ye = res.results[e]["y"]                # [D, C] fp32
        ge = np.where(top[toks] == e, gv[toks], 0.0).sum(axis=1)
        out[toks] += ge[:, None] * ye[:, :len(toks)].T.astype(np.float64)

    return out.astype(np.float32).reshape(B, T, D)


# revision 26
# speedup vs baseline: 1.0477x; 1.0477x over previous
"""MoE block (B=4, T=1024, D=1024, H=4096, E=8, top-2) on 8 Trainium2 cores.

Strategy: expert-parallel. The reference computes every expert for every
token and then keeps only the top-2; the output only depends on the top-2
selections, so we route: host computes the (tiny) gating in float64, core e
runs the dense FFN for expert e over just the tokens routed to it
(transposed layout, zero on-device transposes), and the host combines the
two selected expert outputs per token with the gate weights.

Device kernel per core (C = max tokens routed to any expert, padded):
    h1T[H, C]  = relu(w1[e].T @ xT + b1)   (bf16 in, fp32 accum)
    yT [D, C]  = w2[e].T @ h1T + b2
All matmuls run in bf16 at the full PE rate; PE-roofline is ~C*12288 cycles.
"""

import numpy as np
import ml_dtypes

B, T, D, H, E = 4, 1024, 1024, 4096, 8
TOP_K = 2
N_CORES = 8
KD = D // 128   # 8  K-tiles over D
KH = H // 128   # 32 K-tiles over H

_PROGRAM_CACHE: dict = {}


def _build_program(C: int):
    """Bass/Tile program: dense FFN for one expert over C token columns."""
    if C in _PROGRAM_CACHE:
        return _PROGRAM_CACHE[C]

    from contextlib import ExitStack
    import concourse.bacc as bacc
    import concourse.mybir as mybir
    import concourse.tile as tile

    bf = mybir.dt.bfloat16
    f32 = mybir.dt.float32
    AFT = mybir.ActivationFunctionType

    nc = bacc.Bacc("TRN2", target_bir_lowering=False, debug=False,
                   num_devices=N_CORES)

    A = min(512, C)          # first column block, DMA'd first
    GB = C - A               # remaining columns
    G = 2                    # w1 m-tiles per DMA group
    xa_d = nc.dram_tensor("xa", [128, KD * A], bf, kind="ExternalInput")
    if GB:
        xb_d = nc.dram_tensor("xb", [128, KD * GB], bf, kind="ExternalInput")
    w1_d = nc.dram_tensor("w1g", [KH // G, 128, G * D], bf,
                          kind="ExternalInput")
    w2_d = nc.dram_tensor("w2q", [KD, 128, H], bf, kind="ExternalInput")
    b1_d = nc.dram_tensor("b1t", [128, KH], f32, kind="ExternalInput")
    b2_d = nc.dram_tensor("b2t", [128, KD], f32, kind="ExternalInput")
    y_d = nc.dram_tensor("y", [D, C], f32, kind="ExternalOutput")
    dump_d = nc.dram_tensor("dump", [1, 4], f32, kind="ExternalOutput")

    chunks = [(s, min(512, C - s)) for s in range(0, C, 512)]
    # phase A column passes: pass 0 covers [0,A) from xa, pass 1 covers
    # [A,C) from xb (sub-chunked to <=512-wide matmuls)
    a_passes = [(0, 0, A, [(0, A)] if A else [])]
    if GB:
        a_passes.append((A, 1, GB,
                         [(s, min(512, GB - s)) for s in range(0, GB, 512)]))

    with tile.TileContext(nc) as tc, ExitStack() as ctx:
        pers = ctx.enter_context(tc.tile_pool(name="pers", bufs=1))
        wpool = ctx.enter_context(tc.tile_pool(name="w", bufs=2))
        opool = ctx.enter_context(tc.tile_pool(name="o", bufs=2))
        psum = ctx.enter_context(tc.tile_pool(name="ps", bufs=2, space="PSUM"))
        psum_w = ctx.enter_context(tc.tile_pool(name="psw", bufs=1,
                                                space="PSUM"))

        # Startup-critical DMAs are issued from otherwise-idle engines
        # BEFORE anything else enters those engines' in-order streams:
        # every dma_start costs ~0.6us of serial descriptor processing on
        # its issuing engine, and SyncE additionally has a ~6.5us prologue.
        # The first w1 group goes first so the first matmul can issue as
        # soon as xa lands; xb (the remaining token columns) streams in
        # behind while the first column pass computes.
        # w1 groups and w2 tiles share one slot tag ("wst"): phase-B w2
        # prefetches only become DMA-eligible when late phase-A w1 slots
        # release, which keeps their 1MB transfers out of the startup
        # rush without sem-gated DMA queues (those head-of-line block).
        w1gs = [wpool.tile([128, H], bf, tag="wst", name=f"w1g{g}",
                           bufs=3) for g in range(3)]
        nc.scalar.dma_start(w1gs[0][:, 0:G * D], w1_d[0])
        xa = pers.tile([128, KD * A], bf)
        nc.scalar.dma_start(xa[:], xa_d[:])
        # same scalar desc stream: these trail xa instead of competing
        nc.scalar.dma_start(w1gs[1][:, 0:G * D], w1_d[1])
        nc.scalar.dma_start(w1gs[2][:, 0:G * D], w1_d[2])
        b1s = pers.tile([128, KH], f32)
        nc.gpsimd.dma_start(b1s[:], b1_d[:])
        b2s = pers.tile([128, KD], f32)
        nc.gpsimd.dma_start(b2s[:], b2_d[:])
        xb = pers.tile([128, KD * GB], bf, name="xb") if GB else None

        # PE warmup: dummy matmuls during the initial DMA window keep the
        # PE HAM activity monitor busy so the clock gate opens (1.2 ->
        # 2.4 GHz) before the real matmuls start. The result is copied on
        # the (otherwise idle) vector engine and dumped to a throwaway
        # output so nothing optimizes it away; neither op blocks the DMA
        # descriptor streams above.
        wu = pers.tile([128, 128], bf)
        nc.vector.memset(wu[:], 0.0)
        wacc = psum_w.tile([128, 128], f32, tag="warm")
        for _ in range(36):
            nc.tensor.matmul(wacc[:], wu[:], wu[:], start=True, stop=True)
        wout = pers.tile([1, 4], f32)
        nc.vector.tensor_copy(wout[:], wacc[0:1, 0:4])
        nc.gpsimd.dma_start(dump_d[:], wout[:])

        h1s = [pers.tile([128, C], bf, tag=f"h1_{m}", name=f"h1_{m}")
               for m in range(KH)]

        # Phase A: h1T[m][cols] = relu(w1.T @ xT + b1), column-pass-outer
        # so pass 0 only needs xa. Weights stream in G-m-tile groups and
        # are re-fetched per pass (DMA has bandwidth to spare).
        relu_insts = []
        for (dst_s, src, srcw, subchunks) in a_passes:
            xsrc = xa if src == 0 else xb
            tot = sum(w for _, w in subchunks)
            if src == 1:
                # xb's descriptor sits in the SP stream behind pass-0's
                # slot-gated w1 descriptors, so its 1.2MB flows mid-pass-0
                # -- well before pass 1 needs it, well after the startup
                # rush.
                nc.sync.dma_start(xb[:], xb_d[:])
            for g in range(KH // G):
                if dst_s == 0 and g < 3:
                    w1g = w1gs[g]
                else:
                    w1g = wpool.tile([128, H], bf, tag="wst", bufs=3,
                                     name=f"w1p{dst_s}_{g}")
                    nc.sync.dma_start(w1g[:, 0:G * D], w1_d[g])
                for j in range(G):
                    m = g * G + j
                    acc = psum.tile([128, C], f32, tag="acc")
                    for k in range(KD):
                        for (so, w) in subchunks:
                            nc.tensor.matmul(
                                acc[:, so:so + w],
                                w1g[:, j * D + k * 128:j * D + (k + 1) * 128],
                                xsrc[:, k * srcw + so:k * srcw + so + w],
                                start=(k == 0), stop=(k == KD - 1))
                    relu_insts.append(
                        nc.scalar.activation(h1s[m][:, dst_s:dst_s + tot],
                                             acc[:, 0:tot], AFT.Relu,
                                             bias=b1s[:, m:m + 1]))

        # Phase B: yT[d] = w2.T @ h1T + b2, d over D tiles
        for d in range(KD):
            w2s = wpool.tile([128, H], bf, tag="wst", bufs=3)
            nc.sync.dma_start(w2s[:], w2_d[d])
            acc = psum.tile([128, C], f32, tag="acc")
            last = (d == KD - 1)
            if not last:
                for k in range(KH):
                    for (s, w) in chunks:
                        nc.tensor.matmul(acc[:, s:s + w],
                                         w2s[:, k * 128:(k + 1) * 128],
                                         h1s[k][:, s:s + w],
                                         start=(k == 0), stop=(k == KH - 1))
                ost = opool.tile([128, C], f32, tag="ost")
                nc.scalar.activation(ost[:], acc[:], AFT.Identity,
                                     bias=b2s[:, d:d + 1])
                nc.sync.dma_start(y_d[d * 128:(d + 1) * 128, :], ost[:])
            else:
                # Last tile: finish chunk-by-chunk so the tail ACT+DMA
                # only covers the final chunk instead of the full row.
                for (s, w) in chunks:
                    for k in range(KH):
                        nc.tensor.matmul(acc[:, s:s + w],
                                         w2s[:, k * 128:(k + 1) * 128],
                                         h1s[k][:, s:s + w],
                                         start=(k == 0), stop=(k == KH - 1))
                    ost = opool.tile([128, C], f32, tag="ost")
                    nc.scalar.activation(ost[:, s:s + w], acc[:, s:s + w],
                                         AFT.Identity, bias=b2s[:, d:d + 1])
                    nc.sync.dma_start(y_d[d * 128:(d + 1) * 128, s:s + w],
                                      ost[:, s:s + w])

    nc.compile()
    _PROGRAM_CACHE[C] = nc
    return nc


def _route(x2, gate_w, gate_b):
    """Float64 gating: returns (top_idx [N,2], top_gate [N,2])."""
    logits = x2.astype(np.float64) @ gate_w.astype(np.float64) \
        + gate_b.astype(np.float64)
    z = np.exp(logits - logits.max(axis=1, keepdims=True))
    probs = z / z.sum(axis=1, keepdims=True)
    top = np.argsort(-logits, axis=1, kind="stable")[:, :TOP_K]
    gv = np.take_along_axis(probs, top, axis=1)
    return top, gv


def _pack_weights(w1e, w2e, b1e, b2e, G=2):
    bfl = ml_dtypes.bfloat16
    # w1g[g, p, (j, k, c)] = w1[e][k*128+p, (g*G+j)*128+c]
    w1q = w1e.reshape(KD, 128, KH, 128).transpose(2, 1, 0, 3)  # [m, p, k, c]
    w1g = np.ascontiguousarray(
        w1q.reshape(KH // G, G, 128, D).transpose(0, 2, 1, 3)
        .reshape(KH // G, 128, G * D).astype(bfl))
    w2q = np.ascontiguousarray(
        w2e.reshape(KH, 128, KD, 128).transpose(2, 1, 0, 3)
        .reshape(KD, 128, H).astype(bfl))
    b1t = np.ascontiguousarray(b1e.reshape(KH, 128).T.astype(np.float32))
    b2t = np.ascontiguousarray(b2e.reshape(KD, 128).T.astype(np.float32))
    return w1g, w2q, b1t, b2t


def kernel(x, gate_w, gate_b, w1, b1, w2, b2, _bench_hook=None):
    from concourse.bass_utils import run_bass_kernel_spmd

    bfl = ml_dtypes.bfloat16
    x = np.asarray(x, np.float32)
    x2 = x.reshape(-1, D)                       # [N, D], N = B*T
    N = x2.shape[0]

    top, gv = _route(x2, np.asarray(gate_w), np.asarray(gate_b))

    tok_lists = [np.where((top == e).any(axis=1))[0] for e in range(E)]
    maxload = max(1, max(len(t) for t in tok_lists))
    C = (maxload + 3) // 4 * 4

    nc = _build_program(C)

    x2b = x2.astype(bfl)
    A = min(512, C)
    GB = C - A
    in_maps = []
    for e in range(E):
        toks = tok_lists[e]
        xt = np.zeros((D, C), bfl)
        if len(toks):
            xt[:, :len(toks)] = x2b[toks].T
        xq = xt.reshape(KD, 128, C).transpose(1, 0, 2)   # [p, k, c]
        m = {"xa": np.ascontiguousarray(xq[:, :, :A]).reshape(128, KD * A)}
        if GB:
            m["xb"] = np.ascontiguousarray(xq[:, :, A:]).reshape(128, KD * GB)
        w1g, w2q, b1t, b2t = _pack_weights(
            np.asarray(w1[e], np.float32), np.asarray(w2[e], np.float32),
            np.asarray(b1[e], np.float32), np.asarray(b2[e], np.float32))
        m.update({"w1g": w1g, "w2q": w2q, "b1t": b1t, "b2t": b2t})
        in_maps.append(m)

    res = run_bass_kernel_spmd(nc, in_maps, core_ids=list(range(N_CORES)))
    if _bench_hook is not None:
        _bench_hook(nc, in_maps)

    out = np.zeros((N, D), np.float64)
    for e in range(E):
        toks = tok_lists[e]
        if not len(toks):
            continue
        ye = res.results[e]["y"]                # [D, C] fp32
        ge = np.where(top[toks] == e, gv[toks], 0.0).sum(axis=1)
        out[toks] += ge[:, None] * ye[:, :len(toks)].T.astype(np.float64)

    return out.astype(np.float32).reshape(B, T, D)


# revision 29
# speedup vs baseline: 1.0602x; 1.0119x over previous
"""MoE block (B=4, T=1024, D=1024, H=4096, E=8, top-2) on 8 Trainium2 cores.

Strategy: expert-parallel. The reference computes every expert for every
token and then keeps only the top-2; the output only depends on the top-2
selections, so we route: host computes the (tiny) gating in float64, core e
runs the dense FFN for expert e over just the tokens routed to it
(transposed layout, zero on-device transposes), and the host combines the
two selected expert outputs per token with the gate weights.

Device kernel per core (C = max tokens routed to any expert, padded):
    h1T[H, C]  = relu(w1[e].T @ xT + b1)   (bf16 in, fp32 accum)
    yT [D, C]  = w2[e].T @ h1T + b2
All matmuls run in bf16 at the full PE rate; PE-roofline is ~C*12288 cycles.
"""

import numpy as np
import ml_dtypes

B, T, D, H, E = 4, 1024, 1024, 4096, 8
TOP_K = 2
N_CORES = 8
KD = D // 128   # 8  K-tiles over D
KH = H // 128   # 32 K-tiles over H

_PROGRAM_CACHE: dict = {}


def _build_program(C: int):
    """Bass/Tile program: dense FFN for one expert over C token columns."""
    if C in _PROGRAM_CACHE:
        return _PROGRAM_CACHE[C]

    from contextlib import ExitStack
    import concourse.bacc as bacc
    import concourse.mybir as mybir
    import concourse.tile as tile

    bf = mybir.dt.bfloat16
    f32 = mybir.dt.float32
    AFT = mybir.ActivationFunctionType

    nc = bacc.Bacc("TRN2", target_bir_lowering=False, debug=False,
                   num_devices=N_CORES)

    A = min(512, C)          # first column block, DMA'd first
    GB = C - A               # remaining columns
    G = 2                    # w1 m-tiles per DMA group
    xa_d = nc.dram_tensor("xa", [128, KD * A], bf, kind="ExternalInput")
    if GB:
        xb_d = nc.dram_tensor("xb", [128, KD * GB], bf, kind="ExternalInput")
    w1_d = nc.dram_tensor("w1g", [KH // G, 128, G * D], bf,
                          kind="ExternalInput")
    w2_d = nc.dram_tensor("w2q", [KD, 128, H], bf, kind="ExternalInput")
    b1_d = nc.dram_tensor("b1t", [128, KH], f32, kind="ExternalInput")
    b2_d = nc.dram_tensor("b2t", [128, KD], f32, kind="ExternalInput")
    y_d = nc.dram_tensor("y", [D, C], f32, kind="ExternalOutput")
    dump_d = nc.dram_tensor("dump", [1, 4], f32, kind="ExternalOutput")

    chunks = [(s, min(512, C - s)) for s in range(0, C, 512)]
    # phase A column passes: pass 0 covers [0,A) from xa, pass 1 covers
    # [A,C) from xb (sub-chunked to <=512-wide matmuls)
    a_passes = [(0, 0, A, [(0, A)] if A else [])]
    if GB:
        a_passes.append((A, 1, GB,
                         [(s, min(512, GB - s)) for s in range(0, GB, 512)]))

    with tile.TileContext(nc) as tc, ExitStack() as ctx:
        pers = ctx.enter_context(tc.tile_pool(name="pers", bufs=1))
        wpool = ctx.enter_context(tc.tile_pool(name="w", bufs=2))
        opool = ctx.enter_context(tc.tile_pool(name="o", bufs=2))
        psum = ctx.enter_context(tc.tile_pool(name="ps", bufs=2, space="PSUM"))
        psum_w = ctx.enter_context(tc.tile_pool(name="psw", bufs=1,
                                                space="PSUM"))

        # Startup-critical DMAs are issued from otherwise-idle engines
        # BEFORE anything else enters those engines' in-order streams:
        # every dma_start costs ~0.6us of serial descriptor processing on
        # its issuing engine, and SyncE additionally has a ~6.5us prologue.
        # The first w1 group goes first so the first matmul can issue as
        # soon as xa lands; xb (the remaining token columns) streams in
        # behind while the first column pass computes.
        # w1 groups and w2 tiles share one slot tag ("wst"): phase-B w2
        # prefetches only become DMA-eligible when late phase-A w1 slots
        # release, which keeps their 1MB transfers out of the startup
        # rush without sem-gated DMA queues (those head-of-line block).
        # Each issuing engine's DMAs land on one SW queue at ~0.27MB/us,
        # and queues run in parallel -- so the two startup-critical
        # streams (xa on gpsimd, w1 groups on scalar) get separate queues.
        w1gs = [wpool.tile([128, H], bf, tag="wst", name=f"w1g{g}",
                           bufs=3) for g in range(3)]
        xa = pers.tile([128, KD * A], bf)
        nc.gpsimd.dma_start(xa[:], xa_d[:])
        nc.scalar.dma_start(w1gs[0][:, 0:G * D], w1_d[0])
        nc.scalar.dma_start(w1gs[1][:, 0:G * D], w1_d[1])
        nc.scalar.dma_start(w1gs[2][:, 0:G * D], w1_d[2])
        b1s = pers.tile([128, KH], f32)
        nc.sync.dma_start(b1s[:], b1_d[:])
        b2s = pers.tile([128, KD], f32)
        nc.sync.dma_start(b2s[:], b2_d[:])
        xb = pers.tile([128, KD * GB], bf, name="xb") if GB else None

        # PE warmup: dummy matmuls during the initial DMA window keep the
        # PE HAM activity monitor busy so the clock gate opens (1.2 ->
        # 2.4 GHz) before the real matmuls start. The result is copied on
        # the (otherwise idle) vector engine and dumped to a throwaway
        # output so nothing optimizes it away; neither op blocks the DMA
        # descriptor streams above.
        wu = pers.tile([128, 128], bf)
        nc.vector.memset(wu[:], 0.0)
        wacc = psum_w.tile([128, 128], f32, tag="warm")
        for _ in range(36):
            nc.tensor.matmul(wacc[:], wu[:], wu[:], start=True, stop=True)
        wout = pers.tile([1, 4], f32)
        nc.vector.tensor_copy(wout[:], wacc[0:1, 0:4])
        nc.gpsimd.dma_start(dump_d[:], wout[:])

        h1s = [pers.tile([128, C], bf, tag=f"h1_{m}", name=f"h1_{m}")
               for m in range(KH)]

        # Phase A: h1T[m][cols] = relu(w1.T @ xT + b1), column-pass-outer
        # so pass 0 only needs xa. Weights stream in G-m-tile groups and
        # are re-fetched per pass (DMA has bandwidth to spare).
        relu_insts = []
        for (dst_s, src, srcw, subchunks) in a_passes:
            xsrc = xa if src == 0 else xb
            tot = sum(w for _, w in subchunks)
            if src == 1:
                # xb's descriptor sits in the SP stream behind pass-0's
                # slot-gated w1 descriptors, so its 1.2MB flows mid-pass-0
                # -- well before pass 1 needs it, well after the startup
                # rush.
                nc.sync.dma_start(xb[:], xb_d[:])
            for g in range(KH // G):
                if dst_s == 0 and g < 3:
                    w1g = w1gs[g]
                else:
                    w1g = wpool.tile([128, H], bf, tag="wst", bufs=3,
                                     name=f"w1p{dst_s}_{g}")
                    nc.sync.dma_start(w1g[:, 0:G * D], w1_d[g])
                for j in range(G):
                    m = g * G + j
                    acc = psum.tile([128, C], f32, tag="acc")
                    for k in range(KD):
                        for (so, w) in subchunks:
                            nc.tensor.matmul(
                                acc[:, so:so + w],
                                w1g[:, j * D + k * 128:j * D + (k + 1) * 128],
                                xsrc[:, k * srcw + so:k * srcw + so + w],
                                start=(k == 0), stop=(k == KD - 1))
                    relu_insts.append(
                        nc.scalar.activation(h1s[m][:, dst_s:dst_s + tot],
                                             acc[:, 0:tot], AFT.Relu,
                                             bias=b1s[:, m:m + 1]))

        # Phase B: yT[d] = w2.T @ h1T + b2, d over D tiles
        for d in range(KD):
            w2s = wpool.tile([128, H], bf, tag="wst", bufs=3)
            nc.sync.dma_start(w2s[:], w2_d[d])
            last = (d == KD - 1)
            if not last:
                acc = psum.tile([128, C], f32, tag="acc")
                for k in range(KH):
                    for (s, w) in chunks:
                        nc.tensor.matmul(acc[:, s:s + w],
                                         w2s[:, k * 128:(k + 1) * 128],
                                         h1s[k][:, s:s + w],
                                         start=(k == 0), stop=(k == KH - 1))
                ost = opool.tile([128, C], f32, tag="ost")
                nc.scalar.activation(ost[:], acc[:], AFT.Identity,
                                     bias=b2s[:, d:d + 1])
                nc.sync.dma_start(y_d[d * 128:(d + 1) * 128, :], ost[:])
            else:
                # Last tile: finish chunk-by-chunk (separate PSUM tiles so
                # chunks don't serialize on the ACT drain) so the tail
                # ACT+DMA only covers the final chunk, not the full row.
                for (s, w) in chunks:
                    acc_c = psum.tile([128, C], f32, tag="acc",
                                      name=f"accl{s}")
                    for k in range(KH):
                        nc.tensor.matmul(acc_c[:, 0:w],
                                         w2s[:, k * 128:(k + 1) * 128],
                                         h1s[k][:, s:s + w],
                                         start=(k == 0), stop=(k == KH - 1))
                    ost = opool.tile([128, C], f32, tag="ost")
                    nc.scalar.activation(ost[:, s:s + w], acc_c[:, 0:w],
                                         AFT.Identity, bias=b2s[:, d:d + 1])
                    nc.sync.dma_start(y_d[d * 128:(d + 1) * 128, s:s + w],
                                      ost[:, s:s + w])

    nc.compile()
    _PROGRAM_CACHE[C] = nc
    return nc


def _route(x2, gate_w, gate_b):
    """Float64 gating: returns (top_idx [N,2], top_gate [N,2])."""
    logits = x2.astype(np.float64) @ gate_w.astype(np.float64) \
        + gate_b.astype(np.float64)
    z = np.exp(logits - logits.max(axis=1, keepdims=True))
    probs = z / z.sum(axis=1, keepdims=True)
    top = np.argsort(-logits, axis=1, kind="stable")[:, :TOP_K]
    gv = np.take_along_axis(probs, top, axis=1)
    return top, gv


def _pack_weights(w1e, w2e, b1e, b2e, G=2):
    bfl = ml_dtypes.bfloat16
    # w1g[g, p, (j, k, c)] = w1[e][k*128+p, (g*G+j)*128+c]
    w1q = w1e.reshape(KD, 128, KH, 128).transpose(2, 1, 0, 3)  # [m, p, k, c]
    w1g = np.ascontiguousarray(
        w1q.reshape(KH // G, G, 128, D).transpose(0, 2, 1, 3)
        .reshape(KH // G, 128, G * D).astype(bfl))
    w2q = np.ascontiguousarray(
        w2e.reshape(KH, 128, KD, 128).transpose(2, 1, 0, 3)
        .reshape(KD, 128, H).astype(bfl))
    b1t = np.ascontiguousarray(b1e.reshape(KH, 128).T.astype(np.float32))
    b2t = np.ascontiguousarray(b2e.reshape(KD, 128).T.astype(np.float32))
    return w1g, w2q, b1t, b2t


def kernel(x, gate_w, gate_b, w1, b1, w2, b2, _bench_hook=None):
    from concourse.bass_utils import run_bass_kernel_spmd

    bfl = ml_dtypes.bfloat16
    x = np.asarray(x, np.float32)
    x2 = x.reshape(-1, D)                       # [N, D], N = B*T
    N = x2.shape[0]

    top, gv = _route(x2, np.asarray(gate_w), np.asarray(gate_b))

    tok_lists = [np.where((top == e).any(axis=1))[0] for e in range(E)]
    maxload = max(1, max(len(t) for t in tok_lists))
    C = (maxload + 3) // 4 * 4

    nc = _build_program(C)

    x2b = x2.astype(bfl)
    A = min(512, C)
    GB = C - A
    in_maps = []
    for e in range(E):
        toks = tok_lists[e]
        xt = np.zeros((D, C), bfl)
        if len(toks):
            xt[:, :len(toks)] = x2b[toks].T
        xq = xt.reshape(KD, 128, C).transpose(1, 0, 2)   # [p, k, c]
        m = {"xa": np.ascontiguousarray(xq[:, :, :A]).reshape(128, KD * A)}
        if GB:
            m["xb"] = np.ascontiguousarray(xq[:, :, A:]).reshape(128, KD * GB)
        w1g, w2q, b1t, b2t = _pack_weights(
            np.asarray(w1[e], np.float32), np.asarray(w2[e], np.float32),
            np.asarray(b1[e], np.float32), np.asarray(b2[e], np.float32))
        m.update({"w1g": w1g, "w2q": w2q, "b1t": b1t, "b2t": b2t})
        in_maps.append(m)

    res = run_bass_kernel_spmd(nc, in_maps, core_ids=list(range(N_CORES)))
    if _bench_hook is not None:
        _bench_hook(nc, in_maps)

    out = np.zeros((N, D), np.float64)
    for e in range(E):
        toks = tok_lists[e]
        if not len(toks):
            continue
        ye = res.results[e]["y"]                # [D, C] fp32
        ge = np.where(top[toks] == e, gv[toks], 0.0).sum(axis=1)
        out[toks] += ge[:, None] * ye[:, :len(toks)].T.astype(np.float64)

    return out.astype(np.float32).reshape(B, T, D)
